# revision 22
# baseline (speedup 1.0000x reference)
# BERT self-attention with relation bias (Tableformer) on 8 TRN2 NeuronCores.
#
# Strategy (per core = one batch element, pure data parallelism over B=8):
#   - Q^T/K^T/V projections in bf16 on TensorE (inputs pre-transposed host-side,
#     which is pure layout marshalling; all arithmetic runs on-device).
#   - scores computed TRANSPOSED: S^T[k, q] = sum_d K^T[d,k] * Q^T[d,q] so the
#     attention-mask add and softmax plumbing use per-partition (k) bias slots.
#   - softmax without max-subtraction (scores are O(1) here).
#   - relation bias, two engine paths balanced across heads:
#       * INJ heads (PE path): the additive bias Delta_h[rel[k,q]]
#         (Delta = E[r,h]-E[6,h], zero at r=6) is accumulated INTO the scores
#         PSUM before the exp, via fp8 DoubleRow matmuls: stationary =
#         diag(Delta_{r,h}) pairs, moving = one-hot planes O_r[k,q] in fp8.
#         One DoubleRow matmul injects TWO relation classes at half the
#         per-column cost of a bf16 matmul, so all 6 classes cost 3 matmuls
#         per 512-column tile.  exp() then lands directly in P^T.
#       * DVE heads (ladder path): multiplicative factors after exp:
#         exp(s + E[r,h]) = exp(s) * m_h[r], m_h normalized by m_h[6]; a
#         6-entry table applied as 3 chained custom-DVE "2-entry lookup *
#         multiply" ops on the rel plane (comparing {0,1}, {2,3}, {4,5}),
#         run at FD=4096 to amortize per-op overheads.
#   - ctx^T via a second matmul with P^T as the stationary operand; the softmax
#     denominator comes from a ones-column appended to V (column 64 of V').
#   - normalization (divide by row-sum) on the Pool engine (reciprocal on DVE),
#     written per (head, q-block) straight to HBM to keep SBUF small.
#   - V projection runs before attention while DVE builds the one-hot planes;
#     K/Q projection blocks are interleaved with the first attention heads so
#     PE never sits idle behind the projections.
import os
import sys
import numpy as np

sys.path.insert(0, "/opt/trn_rl_repo")

import concourse.mybir as mybir  # noqa: E402
from concourse import bass, bacc, tile  # noqa: E402
from concourse.bass_utils import run_bass_kernel_spmd  # noqa: E402
from concourse.dve_ops import DveOp, OPS, CUSTOM_DVE_SPECS, get_dve_sub_opcode  # noqa: E402
from concourse.dve_spec import (  # noqa: E402
    Spec, Src0, Src1, C0, C1, One, Zero, select, eq, lower, _has_src1,
)
from concourse.dve_uop import DveOpSpec  # noqa: E402

B, S, D, H, HD, NREL = 8, 1024, 1024, 16, 64, 7
N_CORES = 8
P = 128
NT = S // P  # 8 tiles along any 1024 dim
F32 = mybir.dt.float32
BF16 = mybir.dt.bfloat16
FP8 = mybir.dt.float8e4
I32 = mybir.dt.int32
AF = mybir.ActivationFunctionType
OP = mybir.AluOpType
DR = mybir.MatmulPerfMode.DoubleRow

# Heads processed with the DVE multiplicative ladder; the rest use PE
# DoubleRow bias injection.  Spacing them ~3 apart keeps the DVE ladder
# (~26us/head) overlapped with PE work on the injected heads in between.
_dve_env = os.environ.get("KERNEL_DVE_HEADS", "3,5,9,13")
DVE_HEADS = tuple(int(x) for x in _dve_env.split(",") if x != "")
INJ_HEADS = tuple(h for h in range(H) if h not in DVE_HEADS)

# ---------------------------------------------------------------------------
# Custom DVE ops: out = (in0==a ? s0 : in0==b ? s1 : 1) * in1
# for (a,b) in {(0,1), (2,3), (4,5)} -- all compare the raw rel plane.
# ---------------------------------------------------------------------------
_LUTS = None


def _register_luts():
    global _LUTS
    if _LUTS is not None:
        return _LUTS
    have = {op.name: op for op in OPS}
    if "REL_LUT2_MUL" in have and "REL_LUT2HH_MUL" in have:
        _LUTS = (have["REL_LUT2_MUL"], have["REL_LUT2H_MUL"], have["REL_LUT2HH_MUL"])
        return _LUTS
    two = One + One
    three = two + One
    four = two + two
    five = four + One

    def _mk(a, b, an, bn):
        body = select(eq(Src0, a), C0, select(eq(Src0, b), C1, One)) * Src1

        def _ref(in0, in1, s0, s1, imm2, _an=an, _bn=bn):
            return (
                np.where(in0 == _an, s0, np.where(in0 == _bn, s1, np.float32(1.0)))
                * in1
            )

        return Spec(body=body, reference=_ref)

    import concourse.dve_ops as _dvo
    specs = [
        ("REL_LUT2_MUL", _mk(Zero, One, 0, 1)),
        ("REL_LUT2H_MUL", _mk(two, three, 2, 3)),
        ("REL_LUT2HH_MUL", _mk(four, five, 4, 5)),
    ]
    ops = []
    for name, sp in specs:
        op = DveOp(name, sp, subdim=False, uops_sha={})
        OPS.append(op)
        CUSTOM_DVE_SPECS[op.name] = sp
        _dvo._SUB_OPCODE_FOR_NAME[op.name] = _dvo._CUSTOM_DVE_ROW_BASE + len(OPS) - 1
        assert _dvo._SUB_OPCODE_FOR_NAME[op.name] < 0x20
        for ver in ("v3", "v4"):
            try:
                d = DveOpSpec(
                    name=op.name,
                    opcode=get_dve_sub_opcode(op.name),
                    uops=lower(sp, ver=ver),
                    rd1_en=_has_src1(sp),
                )
                op.uops_sha[ver] = d.sha(ver)
            except Exception:
                pass
        ops.append(op)
    _LUTS = tuple(ops)
    return _LUTS


# ---------------------------------------------------------------------------
# Program builder (runs once per process; input-value independent)
# ---------------------------------------------------------------------------
def _build_program():
    lut_lo, lut_mid, lut_hh = _register_luts()

    nc = bacc.Bacc(
        "TRN2",
        target_bir_lowering=False,
        debug=False,
        enable_asserts=False,
        num_devices=N_CORES,
    )

    # DRAM I/O (per core).  Big tensors arrive pre-cast to bf16 host-side
    # (pure dtype/layout marshalling -- the same rounding the DMA cast would
    # apply) so every load is a cast-free, SP-triggered single DMA.
    xT_d = nc.dram_tensor("xT", [D, S], BF16, kind="ExternalInput")      # hidden[b].T
    wqT_d = nc.dram_tensor("wqT", [D, D], BF16, kind="ExternalInput")    # Wq.T [din, dout]
    wkT_d = nc.dram_tensor("wkT", [D, D], BF16, kind="ExternalInput")
    wvT_d = nc.dram_tensor("wvT", [D, D], BF16, kind="ExternalInput")
    bq_d = nc.dram_tensor("bq", [D], F32, kind="ExternalInput")
    bk_d = nc.dram_tensor("bk", [D], F32, kind="ExternalInput")
    bv_d = nc.dram_tensor("bv", [D], BF16, kind="ExternalInput")
    relT_d = nc.dram_tensor("relT", [S, S], BF16, kind="ExternalInput")  # relation[b].T
    mask_d = nc.dram_tensor("maskv", [S], F32, kind="ExternalInput")     # mask[b,0,0,:]
    remb_d = nc.dram_tensor("relemb", [NREL, H], F32, kind="ExternalInput")
    out_d = nc.dram_tensor("out", [S, D], F32, kind="ExternalOutput")

    from contextlib import ExitStack

    n_inj = len(INJ_HEADS)
    inj_idx = {h: i for i, h in enumerate(INJ_HEADS)}

    with tile.TileContext(nc) as tc, ExitStack() as ctx:
        const = ctx.enter_context(tc.tile_pool(name="const", bufs=1))

        # persistent SBUF tensors (live through attention phase)
        qT = const.tile([P, NT * S], BF16)       # Q^T/8 (+bq/8), dout on partitions
        kT = const.tile([P, NT * S], BF16)       # K^T  (+bk)
        vP = const.tile([P, NT * H * (HD + 1)], BF16)  # V' per seq-block: 16*(64+1)
        rel0 = const.tile([P, NT * S], BF16)     # rel^T as bf16
        # one-hot planes in fp8, paired for DoubleRow: oh[j] holds classes
        # (2j, 2j+1); layout [(kb,jhalf) tile=1024 cols][slot*512 + q]
        oh = [
            const.tile([P, NT * 2 * S], FP8, name=f"oh{j}") for j in range(3)
        ]
        # DoubleRow stationaries: per (inj head, pair): [2 slots x 128] fp8 diag
        ids = const.tile([P, max(n_inj, 1) * 3 * 2 * P], FP8)
        mcols = const.tile([P, NT], F32)         # mask column per k-tile
        bqcols = const.tile([P, NT], F32)        # bq/8 column per dout-block
        bkcols = const.tile([P, NT], F32)
        mprime = const.tile([P, 6 * H], F32)     # exp(E[r,h]-E[6,h])  (DVE heads)
        mraw = const.tile([P, 6 * H], F32)       # E[r,h]-E[6,h]        (INJ heads)
        ones_row = const.tile([1, P], F32)
        ones_row_bf = const.tile([1, P], BF16)
        bv_row2 = const.tile([1, D], BF16)
        ident = const.tile([P, P], BF16)         # identity matrix

        # ---------------- constants prep ----------------
        with (
            tc.tile_pool(name="prep", bufs=2) as prep,
            tc.tile_pool(name="prep_ps", bufs=1, space="PSUM") as prep_ps,
        ):
            nc.sync.dma_start(out=mcols[:], in_=mask_d[:].rearrange("(t p) -> p t", p=P))
            nc.sync.dma_start(out=bqcols[:], in_=bq_d[:].rearrange("(t p) -> p t", p=P))
            nc.sync.dma_start(out=bkcols[:], in_=bk_d[:].rearrange("(t p) -> p t", p=P))
            nc.vector.tensor_scalar_mul(bqcols[:], bqcols[:], 0.125)

            nc.vector.memset(ones_row[:], 1.0)
            nc.vector.memset(ones_row_bf[:], 1.0)

            # identity via affine_select: iota = col - p, keep where == 0
            ones_pp = prep.tile([P, P], BF16)
            nc.vector.memset(ones_pp[:], 1.0)
            nc.gpsimd.affine_select(
                ident[:], ones_pp[:], [[1, P]], OP.is_equal, 0.0,
                base=0, channel_multiplier=-1,
            )

            # rel_emb broadcast to all partitions: [1,112] -> psum [128,112]
            remb_row = prep.tile([1, NREL * H], F32)
            nc.sync.dma_start(
                out=remb_row[:],
                in_=remb_d[:].rearrange("r h -> (r h)").rearrange("(o n) -> o n", o=1),
            )
            mb_ps = prep_ps.tile([P, NREL * H], F32)
            nc.tensor.matmul(mb_ps[:], ones_row[:], remb_row[:])
            mb_sb = prep.tile([P, NREL * H], F32)
            nc.vector.tensor_copy(mb_sb[:], mb_ps[:])
            for r in range(6):
                nc.vector.tensor_tensor(
                    mraw[:, r * H:(r + 1) * H],
                    mb_sb[:, r * H:(r + 1) * H],
                    mb_sb[:, 6 * H:7 * H],
                    OP.subtract,
                )
            nc.scalar.activation(mprime[:], mraw[:], AF.Exp)

            nc.sync.dma_start(out=bv_row2[:], in_=bv_d[:].rearrange("(o d) -> o d", o=1))

            # DoubleRow stationaries: diag(Delta_{2j+slot, h}) in fp8 (DVE).
            # The first two injected heads' stationaries are built before the
            # one-hot planes so the first heads are never blocked on them.
            def emit_ids(hi_range):
                for hi in hi_range:
                    h = INJ_HEADS[hi]
                    for r in range(6):
                        base = ((hi * 3 + r // 2) * 2 + (r % 2)) * P
                        nc.vector.tensor_scalar_mul(
                            ids[:, base:base + P], ident[:],
                            mraw[:, r * H + h: r * H + h + 1],
                        )
            emit_ids(range(min(2, n_inj)))

        # V' gets ones in column 64 of each head slot (Pool memset, early)
        nc.gpsimd.memset(vP[:], 1.0)

        # ---------------- input loads; V projection; one-hot build ----------
        xpool = ctx.enter_context(tc.tile_pool(name="xpool", bufs=1))
        wqk_pool = ctx.enter_context(tc.tile_pool(name="wqk", bufs=2))
        sc_psp = ctx.enter_context(tc.tile_pool(name="sc_ps", bufs=3, space="PSUM"))
        xT = xpool.tile([P, NT * S], BF16)
        w_k0 = wqk_pool.tile([P, NT * P], BF16, tag="w")
        nc.sync.dma_start(
            out=w_k0[:].rearrange("p (t c) -> p t c", c=P),
            in_=wkT_d[:, 0:P].rearrange("(t p) c -> p t c", p=P),
        )
        w_q0 = wqk_pool.tile([P, NT * P], BF16, tag="w")
        nc.scalar.dma_start(
            out=w_q0[:].rearrange("p (t c) -> p t c", c=P),
            in_=wqT_d[:, 0:P].rearrange("(t p) c -> p t c", p=P),
        )
        for t in range(NT):
            nc.sync.dma_start(
                out=xT[:, t * S:(t + 1) * S], in_=xT_d[t * P:(t + 1) * P, :]
            )
        for half in range(2):
            tsl = slice(half * 4, half * 4 + 4)
            nc.scalar.dma_start(
                out=rel0[:].rearrange("p (t s) -> p t s", s=S)[:, tsl, :],
                in_=relT_d[:].rearrange("(t p) s -> p t s", p=P)[:, tsl, :],
            )

        # one-hot fp8 planes from rel0 (DVE, 2x_2p) -- kb-sliced across all
        # (pair, slot) planes so blocks land in the order attention consumes
        rel0_t = rel0[:].rearrange("p (t q) -> p t q", q=512)
        for part in range(4):
            tsl = slice(part * 4, part * 4 + 4)
            for j in range(3):
                ohv = oh[j][:].rearrange("p (t two q) -> p t two q", two=2, q=512)
                for slot in range(2):
                    nc.vector.tensor_scalar(
                        ohv[:, tsl, slot, :], rel0_t[:, tsl, :],
                        float(2 * j + slot), None, OP.is_equal,
                    )
        emit_ids(range(min(2, n_inj), n_inj))

        with tc.tile_pool(name="wvpool", bufs=1) as wvpool:
            wv = wvpool.tile([P, NT * S], BF16)
            nc.scalar.dma_start(
                out=wv[:].rearrange("p (t s) -> p t s", s=S),
                in_=wvT_d[:].rearrange("(t p) s -> p t s", p=P),
            )
            # V natural: lhsT = X^T block, rhs = WvT ; + bv via rank-1 matmul
            for sb in range(NT):
                ps = sc_psp.tile([P, S], F32, tag="scps")
                for kk in range(NT):
                    for j in range(2):
                        nc.tensor.matmul(
                            ps[:, j * 512:(j + 1) * 512],
                            xT[:, kk * S + sb * P: kk * S + (sb + 1) * P],
                            wv[:, kk * S + j * 512: kk * S + (j + 1) * 512],
                            start=(kk == 0),
                            stop=False,
                        )
                for j in range(2):
                    nc.tensor.matmul(
                        ps[:, j * 512:(j + 1) * 512],
                        ones_row_bf[:],
                        bv_row2[:, j * 512:(j + 1) * 512],
                        start=False,
                        stop=True,
                    )
                vslot = vP[:, sb * H * 65:(sb + 1) * H * 65].rearrange(
                    "p (h e) -> p h e", h=H
                )[:, :, 0:HD]
                nc.scalar.activation(
                    vslot, ps[:].rearrange("p (h e) -> p h e", h=H), AF.Copy,
                )

        # ---------------- K/Q projection blocks (interleaved with heads) ----
        def kq_dma(hc, which):
            if hc == 0:
                return w_k0 if which == "k" else w_q0
            wsrc = wkT_d if which == "k" else wqT_d
            w = wqk_pool.tile([P, NT * P], BF16, tag="w", name=f"w_{which}{hc}")
            eng = nc.sync if which == "k" else nc.scalar
            eng.dma_start(
                out=w[:].rearrange("p (t c) -> p t c", c=P),
                in_=wsrc[:, hc * P:(hc + 1) * P].rearrange("(t p) c -> p t c", p=P),
            )
            return w

        def kq_compute(hc, which, w):
            dst = kT if which == "k" else qT
            bias_cols = bkcols if which == "k" else bqcols
            scale = 1.0 if which == "k" else 0.125
            ps = sc_psp.tile([P, S], F32, tag="scps")
            for kk in range(NT):
                for j in range(2):
                    nc.tensor.matmul(
                        ps[:, j * 512:(j + 1) * 512],
                        w[:, kk * P:(kk + 1) * P],
                        xT[:, kk * S + j * 512: kk * S + (j + 1) * 512],
                        start=(kk == 0),
                        stop=(kk == NT - 1),
                    )
            nc.scalar.activation(
                dst[:, hc * S:(hc + 1) * S], ps[:], AF.Identity,
                bias=bias_cols[:, hc:hc + 1], scale=scale,
            )

        kq_compute(0, "k", kq_dma(0, "k"))
        kq_compute(0, "q", kq_dma(0, "q"))

        # ---------------- attention ----------------
        with (
            tc.tile_pool(name="pt", bufs=3) as ptp,
            tc.tile_pool(name="cx_ps", bufs=2, space="PSUM") as cx_psp,
            tc.tile_pool(name="lad", bufs=1) as lad,
            tc.tile_pool(name="rc", bufs=2) as rcp,
            tc.tile_pool(name="st", bufs=2) as stp,
        ):
            from collections import deque

            # Filler pieces: small PE work items (ctx q-blocks, projection
            # halves) interleaved between score tiles so the in-order PE
            # stream always has ready work while ACT drains the exp PSUMs.
            fillers = deque()

            def drain(n):
                for _ in range(n):
                    if fillers:
                        fillers.popleft()()

            def ctx_qb_piece(h, pt, qb):
                def piece():
                    cps = cx_psp.tile([P, HD + 1], F32, tag="cps")
                    for kb in range(NT):
                        nc.tensor.matmul(
                            cps[:],
                            pt[:, kb * S + qb * P: kb * S + (qb + 1) * P],
                            vP[:, kb * H * 65 + h * 65: kb * H * 65 + (h + 1) * 65],
                            start=(kb == 0),
                            stop=(kb == NT - 1),
                        )
                    rc = rcp.tile([P, 1], F32, tag="rc")
                    nc.vector.reciprocal(rc[:], cps[:, HD:HD + 1])
                    stg = stp.tile([P, HD], F32, tag="st")
                    nc.vector.tensor_scalar(stg[:], cps[:, 0:HD], rc[:], None, OP.mult)
                    nc.sync.dma_start(
                        out=out_d[qb * P:(qb + 1) * P, h * HD:(h + 1) * HD],
                        in_=stg[:],
                    )
                return piece

            def queue_ctx(h, pt):
                for qb in range(NT):
                    fillers.append(ctx_qb_piece(h, pt, qb))

            def queue_kq(hc):
                wk_t = kq_dma(hc, "k")
                wq_t = kq_dma(hc, "q")
                fillers.append(lambda: kq_compute(hc, "k", wk_t))
                fillers.append(lambda: kq_compute(hc, "q", wq_t))

            pending = deque()
            for h in range(H):
                off = (h % 2) * HD
                hc = h // 2
                pt = ptp.tile([P, NT * S], BF16, tag="pt")
                if h in DVE_HEADS:
                    # exp on ACT, multiplicative relation ladder on DVE
                    for kb2 in range(NT // 2):
                        ex = lad.tile([P, 2 * S], BF16, tag="ex")
                        t1 = lad.tile([P, 2 * S], BF16, tag="l1")
                        for kh in range(2):
                            kb = kb2 * 2 + kh
                            ps = sc_psp.tile([P, S], F32, tag="scps")
                            for j in range(2):
                                nc.tensor.matmul(
                                    ps[:, j * 512:(j + 1) * 512],
                                    kT[off:off + HD, hc * S + kb * P: hc * S + (kb + 1) * P],
                                    qT[off:off + HD, hc * S + j * 512: hc * S + (j + 1) * 512],
                                )
                            nc.scalar.activation(
                                ex[:, kh * S:(kh + 1) * S], ps[:], AF.Exp,
                                bias=mcols[:, kb:kb + 1], scale=1.0,
                            )
                        kb = kb2 * 2
                        ptk = pt[:, kb * S:(kb + 2) * S]
                        r0 = rel0[:, kb * S:(kb + 2) * S]
                        nc.vector._custom_dve(
                            lut_lo, out=t1[:], in0=r0, in1=ex[:],
                            s0=mprime[:, 0 * H + h: 0 * H + h + 1],
                            s1=mprime[:, 1 * H + h: 1 * H + h + 1],
                        )
                        nc.vector._custom_dve(
                            lut_mid, out=ex[:], in0=r0, in1=t1[:],
                            s0=mprime[:, 2 * H + h: 2 * H + h + 1],
                            s1=mprime[:, 3 * H + h: 3 * H + h + 1],
                        )
                        nc.vector._custom_dve(
                            lut_hh, out=ptk, in0=r0, in1=ex[:],
                            s0=mprime[:, 4 * H + h: 4 * H + h + 1],
                            s1=mprime[:, 5 * H + h: 5 * H + h + 1],
                        )
                        drain(4)
                else:
                    # additive bias injected into the scores PSUM (DoubleRow)
                    hi = inj_idx[h]
                    for kb in range(NT):
                        ps = sc_psp.tile([P, S], F32, tag="scps")
                        for j in range(2):
                            nc.tensor.matmul(
                                ps[:, j * 512:(j + 1) * 512],
                                kT[off:off + HD, hc * S + kb * P: hc * S + (kb + 1) * P],
                                qT[off:off + HD, hc * S + j * 512: hc * S + (j + 1) * 512],
                                start=True, stop=False,
                            )
                            for pj in range(3):
                                base = (hi * 3 + pj) * 2 * P
                                nc.tensor.matmul(
                                    ps[:, j * 512:(j + 1) * 512],
                                    ids[:, base:base + 2 * P].rearrange(
                                        "p (two m) -> p two m", two=2
                                    ),
                                    oh[pj][:, (kb * 2 + j) * S:(kb * 2 + j + 1) * S].rearrange(
                                        "p (two q) -> p two q", two=2
                                    ),
                                    start=False, stop=(pj == 2),
                                    perf_mode=DR,
                                )
                        nc.scalar.activation(
                            pt[:, kb * S:(kb + 1) * S], ps[:], AF.Exp,
                            bias=mcols[:, kb:kb + 1], scale=1.0,
                        )
                        drain(2)

                pending.append((h, pt))
                if len(pending) > 2:
                    queue_ctx(*pending.popleft())
                if h % 2 == 0 and hc + 1 <= NT - 1:
                    queue_kq(hc + 1)

            drain(len(fillers))
            while pending:
                h, pt = pending.popleft()
                for qb in range(NT):
                    ctx_qb_piece(h, pt, qb)()

    nc.compile()
    return nc


_PROGRAM = None


def _get_program():
    global _PROGRAM
    if _PROGRAM is None:
        _PROGRAM = _build_program()
    return _PROGRAM


def _make_in_maps(inputs):
    import ml_dtypes

    bf16 = ml_dtypes.bfloat16
    hidden = np.asarray(inputs["hidden_states"], dtype=np.float32)
    mask = np.asarray(inputs["attention_mask"], dtype=np.float32)
    relation = np.asarray(inputs["relation"], dtype=np.int32)
    wq = np.ascontiguousarray(np.asarray(inputs["Wq"], dtype=np.float32).T).astype(bf16)
    wk = np.ascontiguousarray(np.asarray(inputs["Wk"], dtype=np.float32).T).astype(bf16)
    wv = np.ascontiguousarray(np.asarray(inputs["Wv"], dtype=np.float32).T).astype(bf16)
    bq = np.asarray(inputs["bq"], dtype=np.float32)
    bk = np.asarray(inputs["bk"], dtype=np.float32)
    bv = np.asarray(inputs["bv"], dtype=np.float32).astype(bf16)
    remb = np.asarray(inputs["rel_emb"], dtype=np.float32)

    in_maps = []
    for b in range(N_CORES):
        in_maps.append({
            "xT": np.ascontiguousarray(hidden[b].T).astype(bf16),
            "wqT": wq, "wkT": wk, "wvT": wv,
            "bq": bq, "bk": bk, "bv": bv,
            "relT": np.ascontiguousarray(relation[b].T.astype(np.float32)).astype(bf16),
            "maskv": np.ascontiguousarray(mask[b, 0, 0, :]),
            "relemb": remb,
        })
    return in_maps


LAST_EXEC_NS = None
LAST_RESULTS = None


def kernel(**inputs) -> np.ndarray:
    global LAST_EXEC_NS, LAST_RESULTS
    nc = _get_program()
    in_maps = _make_in_maps(inputs)
    trace = os.environ.get("KERNEL_TRACE", "0") == "1"
    res = run_bass_kernel_spmd(nc, in_maps, list(range(N_CORES)), trace=trace)
    LAST_EXEC_NS = res.exec_time_ns
    LAST_RESULTS = res
    out = np.stack([res.results[b]["out"] for b in range(N_CORES)], axis=0)
    return out.astype(np.float32)


# -------- timing helper: device-resident repeated dispatch --------
def make_bench_fn(inputs):
    """Returns run(M) -> seconds for M back-to-back dispatches (device-resident
    inputs, no donation, block at the end)."""
    import jax
    from jax.sharding import Mesh, PartitionSpec, NamedSharding
    from jax.experimental.shard_map import shard_map
    from concourse import bass2jax
    import concourse.mybir as mb

    nc = _get_program()
    in_maps = _make_in_maps(inputs)
    bass2jax.install_neuronx_cc_hook()

    part_name = nc.partition_id_tensor.name if nc.partition_id_tensor else None
    in_names, out_names, out_avals, zero_outs = [], [], [], []
    for alloc in nc.m.functions[0].allocations:
        if not isinstance(alloc, mb.MemoryLocationSet):
            continue
        name = alloc.memorylocations[0].name
        if alloc.kind == "ExternalInput":
            if name != part_name:
                in_names.append(name)
        elif alloc.kind == "ExternalOutput":
            out_names.append(name)
            shape = tuple(alloc.tensor_shape)
            dtype = mb.dt.np(alloc.dtype)
            out_avals.append(jax.core.ShapedArray(shape, dtype))
            zero_outs.append(np.zeros(shape, dtype))
    n_params = len(in_names)
    all_names = in_names + out_names
    if part_name is not None:
        all_names.append(part_name)

    def _body(*args):
        operands = list(args)
        if part_name is not None:
            operands.append(bass2jax.partition_id_tensor())
        outs = bass2jax._bass_exec_p.bind(
            *operands,
            out_avals=tuple(out_avals),
            in_names=tuple(all_names),
            out_names=tuple(out_names),
            lowering_input_output_aliases=(),
            sim_require_finite=True,
            sim_require_nnan=True,
            nc=nc,
        )
        return tuple(outs)

    devices = jax.devices()[:N_CORES]
    mesh = Mesh(np.asarray(devices), ("core",))
    n_all = n_params + len(out_names)
    sharded = jax.jit(
        shard_map(
            _body, mesh=mesh,
            in_specs=(PartitionSpec("core"),) * n_all,
            out_specs=(PartitionSpec("core"),) * len(out_names),
            check_rep=False,
        ),
        keep_unused=True,
    )
    sh = NamedSharding(mesh, PartitionSpec("core"))
    concat_in = [
        jax.device_put(
            np.concatenate([np.asarray(in_maps[c][nm]) for c in range(N_CORES)], axis=0), sh
        )
        for nm in in_names
    ]
    concat_zeros = [
        jax.device_put(np.zeros((N_CORES * z.shape[0], *z.shape[1:]), z.dtype), sh)
        for z in zero_outs
    ]
    # warmup + compile
    out = sharded(*concat_in, *concat_zeros)
    jax.block_until_ready(out)

    import time

    def run(M):
        t0 = time.perf_counter()
        outs = None
        for _ in range(M):
            outs = sharded(*concat_in, *concat_zeros)
        jax.block_until_ready(outs)
        return time.perf_counter() - t0

    def get_out():
        outs = sharded(*concat_in, *concat_zeros)
        o = np.asarray(outs[0]).reshape(N_CORES, *out_avals[0].shape)
        return o

    run.get_out = get_out
    return run


# -------- simulation helper (single core) for test.py --------
def run_sim_core0(inputs):
    from concourse.bass_interp import CoreSim

    nc = _get_program()
    in_maps = _make_in_maps(inputs)
    sim = CoreSim(nc, trace=False)
    for k, v in in_maps[0].items():
        sim.tensor(k)[:] = v
    sim.simulate(check_with_hw=False)
    return np.array(sim.tensor("out"))
